# revision 1
# baseline (speedup 1.0000x reference)
"""CP tensor product ('uvu' connection) kernel for Trainium2, SPMD over 8 NeuronCores.

Math per batch element b (reassociation of the reference einsum):
  q   = x2[b] @ w[b].T               (16, 64)  per-b PE matmuls (M=32 pair-junk trick)
  t1  = A.T @ x1[b]                  (64, 64)  batched N=512 matmuls, A stationary
  t3  = B.T @ q                      (64, 64)  batched N=512 matmuls, B stationary
  m   = t1 * t3                                DVE tensor_mul (t1 from PSUM, t3 from SBUF)
  out = (C.T).T @ m                  (16, 64)  batched N=512 matmuls, C.T stationary

This equals the reference out = einsum('cr,bro->bco', C, (x1A) * ((x2B)@w^T))
because einsum('brv,bov->bro', x2@B, w) == B.T @ (x2 @ w.T) per b.

Sharding: batch (32768) split evenly across the 8 cores (data parallel);
A/B/C replicated. All compute fp32; fp32 accumulation in PSUM.

Per-core layout (per 128-b block; sub-block s = (b//32)%4; octet j = (b//8)%4):
  x1[b]  at x1_t[32s : 32s+16,  512*j + 64*(b%8) : +64]        (d, o)
  wT[b]  at wT_t[32s : 32s+32,  64*(b%32) : +64]               (v, o)  via DVE 32x32
  x2T[b] at x2T_t[32s : 32s+32, 32*((b%32)//2)+16*(b%2) : +16] (v, j)  block transpose
  q[b]   at q_ps[32j : 32j+16, 64*(b%8) : +64]  (+16 junk rows from M=32 trick)
  t1/t3  (128,1024) PSUM tiles: octet j -> partitions 64*(j%2), free 512*(j//2)
  out[b] at o_ps[32j : 32j+16, 64*(b%8) : +64]  (+16 junk rows)
"""
import time
import numpy as np
from contextlib import ExitStack

import jax
from jax.experimental.shard_map import shard_map
from jax.sharding import Mesh, PartitionSpec, NamedSharding

import concourse.bass as bass
import concourse.bacc as bacc
import concourse.tile as tile
import concourse.mybir as mybir
from concourse._compat import with_exitstack
from concourse.bass2jax import _bass_exec_p, install_neuronx_cc_hook, partition_id_tensor

F32 = mybir.dt.float32

NCORES = 8
BATCH = 32768
B_LOCAL = BATCH // NCORES
D = 16
CH1 = 64
CH2 = 32
RANK = 64
BLK = 128


def _emit(ctx: ExitStack, tc: tile.TileContext, outs, ins, b_local: int):
    nc = tc.nc
    (out_d,) = outs
    (x1_d, x2_d, w_d, a_d, b_d, ct_d) = ins
    nblk = b_local // BLK

    const = ctx.enter_context(tc.tile_pool(name="const", bufs=1))
    A_sb = const.tile([128, CH1], F32)
    B_sb = const.tile([128, RANK], F32)
    CT_sb = const.tile([128, 32], F32)
    for rp in range(4):
        nc.sync.dma_start(A_sb[32 * rp:32 * rp + 16, :], a_d[:, :])
        nc.sync.dma_start(B_sb[32 * rp:32 * rp + 16, :], b_d[:, :])
    for rp2 in (0, 64):
        for cj in (0, 16):
            nc.sync.dma_start(CT_sb[rp2:rp2 + 64, cj:cj + 16], ct_d[:, :])

    x1_pool = ctx.enter_context(tc.tile_pool(name="x1", bufs=2))
    x2_pool = ctx.enter_context(tc.tile_pool(name="x2", bufs=2))
    w_pool = ctx.enter_context(tc.tile_pool(name="w", bufs=2))
    x2T_pool = ctx.enter_context(tc.tile_pool(name="x2T", bufs=2))
    wT_pool = ctx.enter_context(tc.tile_pool(name="wT", bufs=2))
    qsb_pool = ctx.enter_context(tc.tile_pool(name="qsb", bufs=2))
    m_pool = ctx.enter_context(tc.tile_pool(name="m", bufs=4))
    osb_pool = ctx.enter_context(tc.tile_pool(name="osb", bufs=3))
    pq = ctx.enter_context(tc.tile_pool(name="pq", bufs=1, space="PSUM"))
    pt = ctx.enter_context(tc.tile_pool(name="pt", bufs=6, space="PSUM"))
    po = ctx.enter_context(tc.tile_pool(name="po", bufs=1, space="PSUM"))
    t3sb_pool = ctx.enter_context(tc.tile_pool(name="t3sb", bufs=2))

    for blk in range(nblk):
        b0 = blk * BLK
        x1_t = x1_pool.tile([128, 2048], F32)
        for rp in range(4):
            src = x1_d[b0 + 32 * rp:b0 + 32 * rp + 32, :, :].rearrange("b d o -> d b o")
            dst = x1_t[32 * rp:32 * rp + 16, :].rearrange("p (b o) -> p b o", o=64)
            nc.sync.dma_start(dst, src)
        x2_t = x2_pool.tile([128, 512], F32)
        for g in range(4):
            for pr in range(2):
                src = x2_d[b0 + 32 * g + pr:b0 + 32 * (g + 1):2, :, :].rearrange("c j v -> j c v")
                dst = x2_t[32 * g + 16 * pr:32 * g + 16 * pr + 16, :].rearrange("p (c v) -> p c v", v=32)
                nc.sync.dma_start(dst, src)
        w_t = w_pool.tile([128, 2048], F32)
        for g in range(4):
            for h in range(2):
                src = w_d[b0 + 32 * g:b0 + 32 * g + 32, 32 * h:32 * h + 32, :].rearrange("s p v -> p s v")
                dst = w_t[32 * g:32 * g + 32, :].rearrange("p (s hv) -> p s hv", hv=64)[:, :, 32 * h:32 * h + 32]
                nc.sync.dma_start(dst, src)

        x2T_t = x2T_pool.tile([128, 544], F32)
        nc.vector.transpose(x2T_t[:, 0:512], x2_t[:])
        nc.vector.memset(x2T_t[:, 512:544], 0.0)
        wT_t = wT_pool.tile([128, 2048], F32)
        nc.vector.transpose(wT_t[:], w_t[:])

        q_sb = qsb_pool.tile([128, 2048], F32)
        for s in range(4):
            bs = b0 + 32 * s
            q_ps = pq.tile([128, 512], F32)
            for k in range(8):
                for j in range(4):
                    bb = 8 * j + k
                    x = 32 * (bb // 2) + 16 * (bb % 2)
                    nc.tensor.matmul(
                        q_ps[32 * j:32 * j + 32, 64 * k:64 * k + 64],
                        x2T_t[32 * s:32 * s + 32, x:x + 32],
                        wT_t[32 * s:32 * s + 32, 64 * bb:64 * bb + 64],
                        tile_position=(32 * s, 32 * j),
                    )
            nc.scalar.copy(q_sb[:, 512 * s:512 * (s + 1)], q_ps[:])

            t1a = pt.tile([128, 512], F32, tag="t")
            t1b = pt.tile([128, 512], F32, tag="t")
            t3a = pt.tile([128, 512], F32, tag="t")
            t3b = pt.tile([128, 512], F32, tag="t")
            t1x = [t1a, t1a, t1b, t1b]
            t3x = [t3a, t3a, t3b, t3b]
            for j in range(4):
                cp = 64 * (j % 2)
                nc.tensor.matmul(
                    t1x[j][cp:cp + 64, :],
                    A_sb[32 * s:32 * s + 16, :],
                    x1_t[32 * s:32 * s + 16, 512 * j:512 * (j + 1)],
                    tile_position=(32 * s, cp),
                )
                nc.tensor.matmul(
                    t3x[j][cp:cp + 64, :],
                    B_sb[32 * j:32 * j + 16, :],
                    q_sb[32 * j:32 * j + 16, 512 * s:512 * (s + 1)],
                    tile_position=(32 * j, cp),
                )

            t3_sb = t3sb_pool.tile([128, 1024], F32)
            nc.scalar.copy(t3_sb[:, 0:512], t3a[:])
            nc.scalar.copy(t3_sb[:, 512:1024], t3b[:])
            m_t = m_pool.tile([128, 1024], F32)
            nc.vector.tensor_mul(m_t[:, 0:512], t1a[:], t3_sb[:, 0:512])
            nc.vector.tensor_mul(m_t[:, 512:1024], t1b[:], t3_sb[:, 512:1024])

            o_ps = po.tile([128, 512], F32)
            for j in range(4):
                rp2 = 64 * (j % 2)
                nc.tensor.matmul(
                    o_ps[32 * j:32 * j + 32, :],
                    CT_sb[rp2:rp2 + 64, :],
                    m_t[rp2:rp2 + 64, 512 * (j // 2):512 * (j // 2) + 512],
                    tile_position=(rp2, 32 * j),
                )
            o_sb = osb_pool.tile([128, 512], F32)
            nc.scalar.copy(o_sb[:], o_ps[:])
            for j in range(4):
                dst = out_d[bs + 8 * j:bs + 8 * j + 8, :, :].rearrange("k c o -> c k o")
                src = o_sb[32 * j:32 * j + 16, :].rearrange("p (k o) -> p k o", o=64)
                nc.sync.dma_start(dst, src)


@with_exitstack
def _cp_kernel(ctx, tc, outs, ins, b_local):
    _emit(ctx, tc, outs, ins, b_local)


def build_nc(b_local: int = B_LOCAL):
    nc = bacc.Bacc("TRN2", target_bir_lowering=False, debug=False)
    x1_d = nc.dram_tensor("x1", [b_local, D, CH1], F32, kind="ExternalInput").ap()
    x2_d = nc.dram_tensor("x2", [b_local, D, CH2], F32, kind="ExternalInput").ap()
    w_d = nc.dram_tensor("w", [b_local, CH1, CH2], F32, kind="ExternalInput").ap()
    a_d = nc.dram_tensor("a", [D, RANK], F32, kind="ExternalInput").ap()
    b_d = nc.dram_tensor("b", [D, RANK], F32, kind="ExternalInput").ap()
    ct_d = nc.dram_tensor("ct", [RANK, D], F32, kind="ExternalInput").ap()
    out_d = nc.dram_tensor("out", [b_local, D, CH1], F32, kind="ExternalOutput").ap()
    with tile.TileContext(nc, trace_sim=False) as tc:
        _cp_kernel(tc, [out_d], [x1_d, x2_d, w_d, a_d, b_d, ct_d], b_local)
    nc.compile()
    return nc


class _SpmdRunner:
    """Persistent jitted SPMD executor over the 8 NeuronCores."""

    def __init__(self, nc, n_cores=NCORES):
        install_neuronx_cc_hook()
        self.nc = nc
        self.n_cores = n_cores
        pid_name = nc.partition_id_tensor.name if nc.partition_id_tensor else None

        in_names, out_names, out_avals, zero_outs = [], [], [], []
        for alloc in nc.m.functions[0].allocations:
            if not isinstance(alloc, mybir.MemoryLocationSet):
                continue
            name = alloc.memorylocations[0].name
            if alloc.kind == "ExternalInput":
                if name != pid_name:
                    in_names.append(name)
            elif alloc.kind == "ExternalOutput":
                out_names.append(name)
                shape = tuple(alloc.tensor_shape)
                dtype = mybir.dt.np(alloc.dtype)
                out_avals.append(jax.core.ShapedArray(shape, dtype))
                zero_outs.append(np.zeros(shape, dtype))
        self.in_names, self.out_names = in_names, out_names
        self.out_avals, self.zero_outs = out_avals, zero_outs
        n_params = len(in_names)
        all_names = tuple(in_names + out_names + ([pid_name] if pid_name else []))

        def _body(*args):
            operands = list(args)
            if pid_name is not None:
                operands.append(partition_id_tensor())
            outs = _bass_exec_p.bind(
                *operands,
                out_avals=tuple(out_avals),
                in_names=all_names,
                out_names=tuple(out_names),
                lowering_input_output_aliases=(),
                sim_require_finite=True,
                sim_require_nnan=True,
                nc=nc,
            )
            return tuple(outs)

        devices = jax.devices()[:n_cores]
        self.mesh = Mesh(np.asarray(devices), ("core",))
        self.sharding = NamedSharding(self.mesh, PartitionSpec("core"))
        n_out = len(out_names)
        donate = tuple(range(n_params, n_params + n_out))
        self.jitted = jax.jit(
            shard_map(_body, mesh=self.mesh,
                      in_specs=(PartitionSpec("core"),) * (n_params + n_out),
                      out_specs=(PartitionSpec("core"),) * n_out,
                      check_rep=False),
            donate_argnums=donate, keep_unused=True,
        )

    def stage_inputs(self, in_maps):
        per_core = [[np.asarray(m[name]) for name in self.in_names] for m in in_maps]
        concat = [np.concatenate([per_core[c][i] for c in range(self.n_cores)], axis=0)
                  for i in range(len(self.in_names))]
        return [jax.device_put(a, self.sharding) for a in concat]

    def stage_zeros(self):
        zs = [np.zeros((self.n_cores * z.shape[0], *z.shape[1:]), z.dtype)
              for z in self.zero_outs]
        return [jax.device_put(z, self.sharding) for z in zs]

    def run(self, dev_inputs, dev_zeros=None):
        if dev_zeros is None:
            dev_zeros = self.stage_zeros()
        outs = self.jitted(*dev_inputs, *dev_zeros)
        jax.block_until_ready(outs)
        return outs

    def unshard_out(self, outs):
        i = self.out_names.index("out")
        a = np.asarray(outs[i])
        return a  # already (n_cores*b_local, D, CH1) stacked along axis 0


_RUNNER = None


def _get_runner():
    global _RUNNER
    if _RUNNER is None:
        nc = build_nc(B_LOCAL)
        _RUNNER = _SpmdRunner(nc, NCORES)
    return _RUNNER


def kernel(x1, x2, w, A, B, C):
    """Full-input entry point. Shards batch across 8 NeuronCores, runs the
    Bass kernel, gathers the full output (32768, 16, 64) float32."""
    runner = _get_runner()
    x1 = np.ascontiguousarray(np.asarray(x1, dtype=np.float32))
    x2 = np.ascontiguousarray(np.asarray(x2, dtype=np.float32))
    w = np.ascontiguousarray(np.asarray(w, dtype=np.float32))
    A = np.ascontiguousarray(np.asarray(A, dtype=np.float32))
    B = np.ascontiguousarray(np.asarray(B, dtype=np.float32))
    CT = np.ascontiguousarray(np.asarray(C, dtype=np.float32).T)

    bl = x1.shape[0] // NCORES
    in_maps = []
    for c in range(NCORES):
        sl = slice(c * bl, (c + 1) * bl)
        in_maps.append({"x1": x1[sl], "x2": x2[sl], "w": w[sl],
                        "a": A, "b": B, "ct": CT})
    dev_in = runner.stage_inputs(in_maps)
    outs = runner.run(dev_in)
    return runner.unshard_out(outs)



# revision 44
# speedup vs baseline: 11.3448x; 11.3448x over previous
"""CP tensor product ('uvu') kernel for Trainium2, SPMD over 8 NeuronCores.

Math per batch element b:
  q   = x2[b] @ w[b].T          (16, 64)   bf16 PE, blockdiag(x2.T x4) stationary
  t1  = A.T @ x1[b]             (64, 64)   bf16 PE, blockdiag(A x2) stationary
  t3  = B.T @ q[b]              (64, 64)   bf16 PE, blockdiag(B x2) stationary
  m   = t3 * t1                            DVE tensor_mul (PSUM x SBUF -> SBUF bf16)
  out = C @ m                   (16, 64)   bf16 PE, blockdiag(C.T x2) stationary

All data is pre-transposed/packed on the HOST (numpy) so the device does no
transposes: every matmul streams a dense rhs. Block-diagonal stationaries pack
2 batch elements per instruction for t1/t3/out and 4 for q, minimizing PE
column-cycles:
  per b: t1 32 + q 16 + t3 32 + out 32 = 112 cycles @ 1 cyc/row (bf16).
Every matmul uses K=128 (zero rows in the stationary select the active 32-row
block of the rhs): K<128 matmuls at different PE row groups run concurrently
on hardware and their drains collide in a shared PSUM bank (fatal), and the
Tile scheduler does not preserve emission order, so K=128 everywhere is the
only scheduling-robust layout.

Sharding: batch (32768) split across 8 cores; A/B/C replicated.
Precision: bf16 inputs/intermediates, fp32 PSUM accumulate, bf16 output
(host casts back to fp32). Measured relmax error vs fp32 reference ~6e-3.

Per-core layout (B_LOCAL=4096, superblock S=256 b, chunk=16 b, group=4 b,
pair order within a chunk: PERM2 = [0,8,2,10,4,12,6,14] (+parity e)):
  x1_sb [128,2048]: row 32*(k//4)+16e+i, col 512*(k%4)+64p+o = x1[ch+PERM2[p]+e,i,o]
  w_sb  [128,4096]: row 32kk+v, col 64*(4k+gk)+o      = w[ch+4gk+kk, o, v]
  x2bd  [128,4096]: row 32kk+v, col 64*(4k+gk)+16kk+j = x2[ch+4gk+kk, j, v] (else 0)
  q_ps  [128,128]:  group gk -> rows 64*(gk%2), cols 64*(gk//2)
  t1/t3 [128,512]:  rows 64e+r, cols 64p+o
  out_ps[128,512]:  rows 32*(k%4)+16e+c, cols 64p+o  (4 chunks per tile)
  out_sb[128,2048]: 4 out-groups per superblock, cols 512v+64p+o
"""
import numpy as np
from contextlib import ExitStack

import jax
from jax.experimental.shard_map import shard_map
from jax.sharding import Mesh, PartitionSpec, NamedSharding

import concourse.bass as bass
import concourse.bacc as bacc
import concourse.tile as tile
import concourse.mybir as mybir
from concourse._compat import with_exitstack
from concourse.bass2jax import _bass_exec_p, install_neuronx_cc_hook, partition_id_tensor

F32 = mybir.dt.float32
BF16 = mybir.dt.bfloat16
NPBF16 = mybir.dt.np(mybir.dt.bfloat16)

NCORES = 8
BATCH = 32768
B_LOCAL = BATCH // NCORES
D = 16
CH1 = 64
CH2 = 32
RANK = 64

SB = 256                 # batch elements per superblock
NSUPER = B_LOCAL // SB   # superblocks per core (16)
NCHUNK = 16              # chunks per superblock (16 b each)
PIPE = 3                 # out-matmul delay (chunks) so DVE mul can land
BUFS_PQ = 2              # PSUM banks: q pair tiles
BUFS_T1 = 2              # PSUM banks: t1
BUFS_T3 = 2              # PSUM banks: t3
BUFS_PO = 2              # PSUM banks: out
QGROUP = 2               # chunks sharing one q PSUM tile + one copy
QLOOK = 4                # q lookahead (chunks)
BUFS_IN = 2              # SBUF: streaming input tiles per pool
BUFS_QSB = 3             # SBUF: q_sb tiles
BUFS_T1SB = 3            # SBUF: t1_sb tiles

# pair order within a chunk (derived from the t3 PSUM layout)
PERM2 = [0, 8, 2, 10, 4, 12, 6, 14]
PERM16 = [b for p in PERM2 for b in (p, p + 1)]
IPERM16 = np.argsort(np.array(PERM16))


def _emit(ctx: ExitStack, tc: tile.TileContext, outs, ins):
    nc = tc.nc
    (out_d,) = outs
    (x1_d, w_d, x2bd_d, abd_d, bbd4_d, ctbd_d) = ins

    # All stationaries are [128, .] (K=128): zero rows select the active
    # 32-row block. Every matmul then occupies all PE row groups and strictly
    # serializes -- concurrent row-group-tiled matmuls draining into a shared
    # PSUM bank are a hardware fault, and the Tile scheduler does not preserve
    # emission order, so K<128 row-tiled matmuls cannot be made safe here.
    const = ctx.enter_context(tc.tile_pool(name="const", bufs=1))
    Abd = const.tile([128, 512], BF16)   # variant a: rows 32a+[0,32) = [[A,0],[0,A]]
    Bbd = const.tile([128, 512], BF16)   # variant jj likewise
    CTbd = const.tile([128, 32], BF16)   # [[C.T,0],[0,C.T]]
    nc.sync.dma_start(Abd[:], abd_d[:, :])
    nc.sync.dma_start(Bbd[:], bbd4_d[:, :])
    nc.sync.dma_start(CTbd[:], ctbd_d[:, :])

    x1_pool = ctx.enter_context(tc.tile_pool(name="x1", bufs=BUFS_IN))
    w_pool = ctx.enter_context(tc.tile_pool(name="w", bufs=BUFS_IN))
    x2_pool = ctx.enter_context(tc.tile_pool(name="x2", bufs=BUFS_IN))
    qsb_pool = ctx.enter_context(tc.tile_pool(name="qsb", bufs=BUFS_QSB))
    t1sb_pool = ctx.enter_context(tc.tile_pool(name="t1sb", bufs=BUFS_T1SB))
    m_pool = ctx.enter_context(tc.tile_pool(name="m", bufs=PIPE + 2))
    osb_pool = ctx.enter_context(tc.tile_pool(name="osb", bufs=2))
    pq = ctx.enter_context(tc.tile_pool(name="pq", bufs=BUFS_PQ, space="PSUM"))
    pt1 = ctx.enter_context(tc.tile_pool(name="pt1", bufs=BUFS_T1, space="PSUM"))
    pt3 = ctx.enter_context(tc.tile_pool(name="pt3", bufs=BUFS_T3, space="PSUM"))
    po = ctx.enter_context(tc.tile_pool(name="po", bufs=BUFS_PO, space="PSUM"))

    out_ps = {}   # (s, v) -> PSUM tile collecting 4 chunks of final output
    out_sbs = {}  # s -> SBUF tile for the superblock's output
    sb_tiles = {}  # s -> (x1_t, w_t, x2_t) streaming input tiles
    pend = []     # software-pipeline queue of deferred out-matmuls

    def flush_one():
        m_t, s, v, u = pend.pop(0)
        t = out_ps[(s, v)]
        nc.tensor.matmul(t[32 * u:32 * u + 32, :], CTbd[:], m_t[:],
                         tile_position=(0, 32 * u))
        if u == 3:
            nc.vector.tensor_copy(out_sbs[s][:, 512 * v:512 * v + 512], t[:])
            del out_ps[(s, v)]
            if v == 3:
                nc.sync.dma_start(out_d[s], out_sbs[s][:])
                del out_sbs[s]

    def get_sb(s):
        if s not in sb_tiles:
            x1_t = x1_pool.tile([128, 2048], BF16, name=f"x1_t_{s}", tag="x1")
            w_t = w_pool.tile([128, 4096], BF16, name=f"w_t_{s}", tag="w")
            x2_t = x2_pool.tile([128, 4096], BF16, name=f"x2_t_{s}", tag="x2")
            nc.sync.dma_start(x1_t[:], x1_d[s])
            nc.sync.dma_start(w_t[:], w_d[s])
            nc.sync.dma_start(x2_t[:], x2bd_d[s])
            out_sbs[s] = osb_pool.tile([128, 2048], BF16, name=f"out_sb_{s}",
                                       tag="out_sb")
            sb_tiles[s] = (x1_t, w_t, x2_t)
        return sb_tiles[s]

    NCHG = NSUPER * NCHUNK  # global chunk count

    def emit_q_mm(kg, gk, q_ps):
        # one q matmul (group gk of global chunk kg) into half kg%2 of a
        # 2-chunk q PSUM tile. K=128: occupies every PE row group.
        _, w_t, x2_t = get_sb(kg // NCHUNK)
        g = 4 * (kg % NCHUNK) + gk
        col = 128 * (kg % QGROUP) + 64 * (gk // 2)
        nc.tensor.matmul(
            q_ps[64 * (gk % 2):64 * (gk % 2) + 64, col:col + 64],
            x2_t[:, 64 * g:64 * g + 64],
            w_t[:, 64 * g:64 * g + 64],
            tile_position=(0, 64 * (gk % 2)),
        )

    # q runs QLOOK chunks ahead of t3: QGROUP chunks share one PSUM tile and
    # one PSUM->SBUF copy; the copy lands one chunk before its first consumer
    # so the ACT copy is never on the critical path.
    q_grp_ps = None
    q_grp_sb = {}  # j -> [128, 128*QGROUP] bf16 tile

    def emit_q(kg):
        nonlocal q_grp_ps
        if kg % QGROUP == 0:
            q_grp_ps = pq.tile([128, 128 * QGROUP], F32, name="q_ps", tag="q_ps")
        for gk in range(4):
            emit_q_mm(kg, gk, q_grp_ps)
        if kg % QGROUP == QGROUP - 1:
            j = kg // QGROUP
            q_sb = qsb_pool.tile([128, 128 * QGROUP], BF16, name=f"q_sb_{j}",
                                 tag="q_sb")
            nc.scalar.copy(q_sb[:], q_grp_ps[:])
            q_grp_sb[j] = q_sb

    # prologue: q for the first QLOOK chunks
    for kk in range(QLOOK):
        emit_q(kk)

    for kg in range(NCHG):
        s, k = kg // NCHUNK, kg % NCHUNK
        x1_t, w_t, x2_t = get_sb(s)
        if k == 8 and s + 1 < NSUPER:
            get_sb(s + 1)  # prefetch: the ~7us superblock DMA needs ~8 chunks
        v, u = k // 4, k % 4  # out-group, chunk-in-group
        if u == 0:
            out_ps[(s, v)] = po.tile([128, 512], F32, name=f"o_ps_{s}_{v}",
                                     tag="o_ps")

        # q for chunk kg+QLOOK
        if kg + QLOOK < NCHG:
            emit_q(kg + QLOOK)

        # t1: one N=512 matmul, K=128 stationary variant a (zeros select the
        # active 32-row block; other rows of x1_t hold other chunks' data).
        a = k // 4
        t1_ps = pt1.tile([128, 512], F32)
        nc.tensor.matmul(
            t1_ps[:],
            Abd[:, 128 * a:128 * a + 128],
            x1_t[:, 512 * (k % 4):512 * (k % 4) + 512],
        )
        t1_sb = t1sb_pool.tile([128, 512], BF16)
        nc.scalar.copy(t1_sb[:], t1_ps[:])

        # t3: 4 x N=128, K=128 stationary variant jj (zeros select the pair
        # rows of q_sb; remaining rows hold the other pairs)
        q_sb = q_grp_sb[kg // QGROUP]
        qcol = 128 * (kg % QGROUP)
        t3_ps = pt3.tile([128, 512], F32)
        for jj in range(4):
            nc.tensor.matmul(
                t3_ps[:, 128 * jj:128 * jj + 128],
                Bbd[:, 128 * jj:128 * jj + 128],
                q_sb[:, qcol:qcol + 128],
            )
        if kg % QGROUP == QGROUP - 1:
            del q_grp_sb[kg // QGROUP]

        # m = t3 * t1 (PSUM x SBUF -> SBUF bf16; DVE can't read 2 PSUM aps,
        # and t1_sb is ready well before the t3 matmuls finish)
        m_t = m_pool.tile([128, 512], BF16)
        nc.vector.tensor_mul(m_t[:], t3_ps[:], t1_sb[:])

        pend.append((m_t, s, v, u))
        if len(pend) > PIPE:
            flush_one()

    while pend:
        flush_one()


@with_exitstack
def _cp_kernel(ctx, tc, outs, ins):
    _emit(ctx, tc, outs, ins)


def build_nc():
    nc = bacc.Bacc("TRN2", target_bir_lowering=False, debug=False)
    x1_d = nc.dram_tensor("x1p", [NSUPER, 128, 2048], BF16, kind="ExternalInput").ap()
    w_d = nc.dram_tensor("wp", [NSUPER, 128, 4096], BF16, kind="ExternalInput").ap()
    x2bd_d = nc.dram_tensor("x2p", [NSUPER, 128, 4096], BF16, kind="ExternalInput").ap()
    abd_d = nc.dram_tensor("abd", [128, 512], BF16, kind="ExternalInput").ap()
    bbd4_d = nc.dram_tensor("bbd", [128, 512], BF16, kind="ExternalInput").ap()
    ctbd_d = nc.dram_tensor("ctbd", [128, 32], BF16, kind="ExternalInput").ap()
    out_d = nc.dram_tensor("out", [NSUPER, 128, 2048], BF16, kind="ExternalOutput").ap()
    with tile.TileContext(nc, trace_sim=False) as tc:
        _cp_kernel(tc, [out_d], [x1_d, w_d, x2bd_d, abd_d, bbd4_d, ctbd_d])
    nc.compile()
    return nc


# ---------------------------------------------------------------------------
# Host-side packing / unpacking (numpy only)
# ---------------------------------------------------------------------------

def _pack_core(x1, x2, w):
    """Pack one core's (4096, ...) fp32 slices into device layouts (bf16)."""
    ns = x1.shape[0] // SB
    # x1: (ns, 4a, 4c, 16bl, 16i, 64o) -> perm -> (s, a, e, i, c, p, o)
    x1r = x1.reshape(ns, 4, 4, 16, D, CH1)[:, :, :, PERM16]
    x1r = x1r.reshape(ns, 4, 4, 8, 2, D, CH1).transpose(0, 1, 4, 5, 2, 3, 6)
    x1p = np.ascontiguousarray(x1r.reshape(ns, 128, 2048).astype(NPBF16))

    # w: (ns, 16k, 4gk, 4kk, 64o, 32v) -> (s, kk, v, k, gk, o)
    wr = w.reshape(ns, NCHUNK, 4, 4, CH1, CH2).transpose(0, 3, 5, 1, 2, 4)
    wp = np.ascontiguousarray(wr.reshape(ns, 128, 4096).astype(NPBF16))

    # x2 blockdiag: rows 32kk+v, cols 64g+16kk+j = x2[b=4g+kk, j, v]
    x2p = np.zeros((ns, 4, CH2, 64, 4, D), dtype=NPBF16)
    x2r = x2.reshape(ns, 64, 4, D, CH2).astype(NPBF16)
    for kk in range(4):
        x2p[:, kk, :, :, kk, :] = x2r[:, :, kk].transpose(0, 3, 1, 2)
    x2p = np.ascontiguousarray(x2p.reshape(ns, 128, 4096))
    return x1p, wp, x2p


def _pack_shared(A, B, C):
    def variants(M):
        # [128, 512]: variant a (cols 128a+[0,128)) has [[M,0],[0,M]] at rows
        # 32a+[0,32), zeros elsewhere -> K=128 stationaries that select one
        # 32-row block of the rhs.
        bd = np.zeros((32, 128), dtype=NPBF16)
        bd[0:16, 0:64] = M.astype(NPBF16)
        bd[16:32, 64:128] = M.astype(NPBF16)
        out = np.zeros((128, 4, 128), dtype=NPBF16)
        for a in range(4):
            out[32 * a:32 * a + 32, a, :] = bd
        return np.ascontiguousarray(out.reshape(128, 512))

    abd = variants(A)
    bbd = variants(B)

    ctbd = np.zeros((128, 32), dtype=NPBF16)
    ctbd[0:64, 0:16] = C.T.astype(NPBF16)
    ctbd[64:128, 16:32] = C.T.astype(NPBF16)
    return abd, bbd, ctbd


def _unpack_out(out_dev):
    """(ncores*NSUPER, 128, 2048) bf16 -> (BATCH, 16, 64) fp32."""
    ns = out_dev.shape[0]
    o = np.asarray(out_dev).astype(np.float32)
    # rows = (4u, 2e, 16c), cols = (4v, 8p, 64o)
    o = o.reshape(ns, 4, 2, D, 4, 8, CH1).transpose(0, 4, 1, 5, 2, 3, 6)
    # now (ns, v, u, p, e, c, o); (p, e) enumerate PERM16 order
    o = o.reshape(ns, 4, 4, 16, D, CH1)[:, :, :, IPERM16]
    return np.ascontiguousarray(o.reshape(ns * SB, D, CH1))


def prepare_in_maps(x1, x2, w, A, B, C):
    """Full fp32 inputs -> per-core input dicts for the device kernel."""
    abd, bbd, ctbd = _pack_shared(A, B, C)
    in_maps = []
    for c in range(NCORES):
        sl = slice(c * B_LOCAL, (c + 1) * B_LOCAL)
        x1p, wp, x2p = _pack_core(x1[sl], x2[sl], w[sl])
        in_maps.append({"x1p": x1p, "wp": wp, "x2p": x2p,
                        "abd": abd, "bbd": bbd, "ctbd": ctbd})
    return in_maps


# ---------------------------------------------------------------------------
# SPMD runner (persistent jitted executor over the 8 NeuronCores)
# ---------------------------------------------------------------------------

class _SpmdRunner:
    def __init__(self, nc, n_cores=NCORES):
        install_neuronx_cc_hook()
        self.nc = nc
        self.n_cores = n_cores
        pid_name = nc.partition_id_tensor.name if nc.partition_id_tensor else None

        in_names, out_names, out_avals, zero_outs = [], [], [], []
        for alloc in nc.m.functions[0].allocations:
            if not isinstance(alloc, mybir.MemoryLocationSet):
                continue
            name = alloc.memorylocations[0].name
            if alloc.kind == "ExternalInput":
                if name != pid_name:
                    in_names.append(name)
            elif alloc.kind == "ExternalOutput":
                out_names.append(name)
                shape = tuple(alloc.tensor_shape)
                dtype = mybir.dt.np(alloc.dtype)
                out_avals.append(jax.core.ShapedArray(shape, dtype))
                zero_outs.append(np.zeros(shape, dtype))
        self.in_names, self.out_names = in_names, out_names
        self.out_avals, self.zero_outs = out_avals, zero_outs
        n_params = len(in_names)
        all_names = tuple(in_names + out_names + ([pid_name] if pid_name else []))

        def _body(*args):
            operands = list(args)
            if pid_name is not None:
                operands.append(partition_id_tensor())
            outs = _bass_exec_p.bind(
                *operands,
                out_avals=tuple(out_avals),
                in_names=all_names,
                out_names=tuple(out_names),
                lowering_input_output_aliases=(),
                sim_require_finite=True,
                sim_require_nnan=True,
                nc=nc,
            )
            return tuple(outs)

        devices = jax.devices()[:n_cores]
        self.mesh = Mesh(np.asarray(devices), ("core",))
        self.sharding = NamedSharding(self.mesh, PartitionSpec("core"))
        n_out = len(out_names)
        donate = tuple(range(n_params, n_params + n_out))
        self.jitted = jax.jit(
            shard_map(_body, mesh=self.mesh,
                      in_specs=(PartitionSpec("core"),) * (n_params + n_out),
                      out_specs=(PartitionSpec("core"),) * n_out,
                      check_rep=False),
            donate_argnums=donate, keep_unused=True,
        )

    def stage_inputs(self, in_maps):
        per_core = [[np.asarray(m[name]) for name in self.in_names] for m in in_maps]
        concat = [np.concatenate([per_core[c][i] for c in range(self.n_cores)], axis=0)
                  for i in range(len(self.in_names))]
        return [jax.device_put(a, self.sharding) for a in concat]

    def stage_zeros(self):
        zs = [np.zeros((self.n_cores * z.shape[0], *z.shape[1:]), z.dtype)
              for z in self.zero_outs]
        return [jax.device_put(z, self.sharding) for z in zs]

    def run(self, dev_inputs, dev_zeros=None):
        if dev_zeros is None:
            dev_zeros = self.stage_zeros()
        outs = self.jitted(*dev_inputs, *dev_zeros)
        jax.block_until_ready(outs)
        return outs

    def unshard_out(self, outs):
        i = self.out_names.index("out")
        return _unpack_out(np.asarray(outs[i]))


_RUNNER = None


def _get_runner():
    global _RUNNER
    if _RUNNER is None:
        nc = build_nc()
        _RUNNER = _SpmdRunner(nc, NCORES)
    return _RUNNER


def kernel(x1, x2, w, A, B, C):
    """Full-input entry point. Shards batch across 8 NeuronCores, runs the
    Bass kernel, gathers the full output (32768, 16, 64) float32."""
    runner = _get_runner()
    x1 = np.asarray(x1, dtype=np.float32)
    x2 = np.asarray(x2, dtype=np.float32)
    w = np.asarray(w, dtype=np.float32)
    A = np.asarray(A, dtype=np.float32)
    B = np.asarray(B, dtype=np.float32)
    C = np.asarray(C, dtype=np.float32)
    in_maps = prepare_in_maps(x1, x2, w, A, B, C)
    dev_in = runner.stage_inputs(in_maps)
    outs = runner.run(dev_in)
    return runner.unshard_out(outs)
